# revision 14
# baseline (speedup 1.0000x reference)
"""FBGCN layer kernel for 8 Trainium2 NeuronCores.

out = aL * GCNConv(x, edge_index; W_conv, b_conv) + aH * (Lsym @ relu(x @ W_high.T))

Sharding: 1D row-partition of output nodes across 8 cores (1536 rows each),
with a host-chosen permutation of targets so that every 32-target group has
(nearly) the same number of incoming edges on every core. Host unpermutes
the concatenated result.

Each core:
  - computes Y = relu(x @ W_high.T)/s and xw = x @ W_conv.T for ALL nodes
    (x replicated, columns permuted per-core), writes xw to a DRAM scratch
    in a partition-contiguous layout,
  - streams its column slice of clip(s*aH*Lsym.T) (fp8 e3m4) through the PE
    with Y blocks stationary, accumulating HhT = (aH*Lsym_rows @ Y).T in
    PSUM (the 1/s is folded into W_high),
  - gathers per-edge source rows of xw from the scratch with dma_gather
    (one gather per 32-target group, exact counts - no padding), multiplies
    by a host-built sparse "segment matrix" (aL*norm weights folded in) on
    the PE to get the GCN edge aggregation per 128-target block (interleaved
    with the Lsym stream); self-loop terms are applied on-chip from xw
    directly (the per-core permutation puts own targets at scratch blocks
    0..MB-1),
  - transposes HhT blocks on the PE, adds everything and writes
    out[1536, 64] fp32.
No cross-core communication is needed.
"""

import numpy as np

import concourse.bacc as bacc
import concourse.mybir as mybir
import concourse.tile as tile
from concourse.bass_utils import run_bass_kernel_spmd

N, E, D = 12288, 196608, 64
NCORES = 8
M = N // NCORES          # 1536 output rows per core
MB = M // 128            # 12 target blocks per core
KB = N // 128            # 96 contraction blocks
ZERO_ROW = N             # scratch row of zeros (dummy gather target)
SCR_ROWS = N + 1
XT_CHUNK = 48            # kb blocks per xT DMA chunk (one load)
G = 32                   # targets per gather-group
GPB = 128 // G           # groups per 128-target block

DT = mybir.dt.float16
NPDT = np.float16
F32 = mybir.dt.float32
AFT = mybir.ActivationFunctionType
ALU = mybir.AluOpType

# high-pass (Lsym) stream dtype: "fp8" (e3m4, scaled) or "fp16"
LS_MODE = "fp8"
LS_SCALE = 512.0
E3M4_MAX = 15.5


def _ls_dt():
    return mybir.dt.float8e3 if LS_MODE == "fp8" else DT


def _scratch_row(pos):
    """xT position -> scratch row (partition-contiguous layout)."""
    return (pos % 128) * KB + pos // 128


def _build_program(C: int, NI: int, ls_mode: str, ls_pack=8, ls_bufs=3,
                   xt_chunk=XT_CHUNK, gcn_sched=None, mm_w=512,
                   out_split=2):
    """Build the SPMD Bass program.

    C = 128-slot chunks per G-target group; NI = gather index count per
    group (static, uniform across cores; NI <= C*128)."""
    QB = GPB * C            # chunks per 128-target block
    S = MB * QB * 128       # slot space per core (gidx layout)
    lsdt = mybir.dt.float8e3 if ls_mode == "fp8" else DT
    lsz = 1 if ls_mode == "fp8" else 2
    # keep SBUF in budget for denser graphs (fallback path)
    if C > 5:
        ls_bufs = 2
    if C > 8:
        ls_pack = 4
    nc = bacc.Bacc("TRN2", target_bir_lowering=False, debug=False,
                   num_devices=NCORES)

    lsymT = nc.dram_tensor("lsymT", [N, M], lsdt, kind="ExternalInput")
    xT = nc.dram_tensor("xT", [D, N], DT, kind="ExternalInput")
    wt2 = nc.dram_tensor("wt2", [D, 2 * D], DT, kind="ExternalInput")
    segT = nc.dram_tensor("segT", [MB * 128, QB * G], DT,
                          kind="ExternalInput")
    gidx = nc.dram_tensor("gidx", [128, S // 16], mybir.dt.int16,
                          kind="ExternalInput")
    bias128 = nc.dram_tensor("bias128", [128, D], F32, kind="ExternalInput")
    wself = nc.dram_tensor("wself", [128, MB], F32, kind="ExternalInput")
    ident = nc.dram_tensor("ident", [D, D], F32, kind="ExternalInput")
    outp = nc.dram_tensor("out", [M, D], F32, kind="ExternalOutput")

    with tile.TileContext(nc) as tc:
        with (
            tc.tile_pool(name="consts", bufs=1) as consts,
            tc.tile_pool(name="dram", bufs=1, space="DRAM") as dram,
            tc.tile_pool(name="xt", bufs=2) as xt_pool,
            tc.tile_pool(name="ls", bufs=ls_bufs) as ls_pool,
            tc.tile_pool(name="seg", bufs=2) as seg_pool,
            tc.tile_pool(name="msg", bufs=2) as msg_pool,
            tc.tile_pool(name="msgh", bufs=2) as msgh_pool,
            tc.tile_pool(name="hlp", bufs=2) as hlp_pool,
            tc.tile_pool(name="psb", bufs=1, space="PSUM") as ps_big,
            tc.tile_pool(name="psa", bufs=2, space="PSUM") as ps_a0,
            tc.tile_pool(name="pss", bufs=3, space="PSUM") as ps_small,
        ):
            # ---- constants / persistent tiles ----
            wt2_sb = consts.tile([D, 2 * D], DT, tag="wt2")
            nc.scalar.dma_start(wt2_sb[:], wt2[:])
            ident_sb = consts.tile([D, D], F32, tag="ident")
            nc.scalar.dma_start(ident_sb[:], ident[:])
            bias_sb = consts.tile([128, D], F32, tag="bias")
            nc.scalar.dma_start(bias_sb[:], bias128[:])
            wself_sb = consts.tile([128, MB], F32, tag="wself")
            nc.scalar.dma_start(wself_sb[:], wself[:])
            idx_sb = consts.tile([128, S // 16], mybir.dt.int16, tag="idx")
            nc.scalar.dma_start(idx_sb[:], gidx[:])
            zrow_sb = consts.tile([1, D], F32, tag="zrow")
            nc.vector.memset(zrow_sb[:], 0)
            y_all = consts.tile([128, KB * D], DT, tag="yall")
            xw_all = consts.tile([128, KB * D], F32, tag="xwall")
            hh_sb = consts.tile([D, M], F32, tag="hh")
            hl_sb = consts.tile([128, MB * D], F32, tag="hl")
            ob_sb = consts.tile([128, MB * D], F32, tag="ob")

            scratch = dram.tile([SCR_ROWS, D], F32, tag="scr")
            nc.scalar.dma_start(scratch[ZERO_ROW:ZERO_ROW + 1, :], zrow_sb[:])

            # ---- phase A0: Y = relu(x@Wh.T), xw = x@Wc.T for all nodes ----
            # four kb blocks per PSUM tile / activation / copy
            xt_sb = None
            for kb4 in range(KB // 4):
                kb = kb4 * 4
                if kb % xt_chunk == 0:
                    xt_sb = xt_pool.tile([D, xt_chunk * 128], DT, tag="xt")
                    c0 = kb * 128
                    nc.scalar.dma_start(xt_sb[:], xT[:, c0:c0 + xt_chunk * 128])
                ps = ps_a0.tile([128, 8 * D], F32, tag="ps")
                for h in range(4):
                    o = (kb + h) % xt_chunk
                    nc.tensor.matmul(
                        ps[:, h * 2 * D:(h + 1) * 2 * D],
                        lhsT=xt_sb[:, o * 128:(o + 1) * 128],
                        rhs=wt2_sb[:],
                        start=True, stop=True,
                    )
                psv = ps[:].rearrange("p (h c) -> p h c", h=4)
                nc.scalar.activation(
                    y_all[:, kb * D:(kb + 4) * D]
                    .rearrange("p (h c) -> p h c", h=4),
                    psv[:, :, 0:D], AFT.Relu)
                nc.vector.tensor_copy(
                    xw_all[:, kb * D:(kb + 4) * D]
                    .rearrange("p (h c) -> p h c", h=4),
                    psv[:, :, D:2 * D])
            # one partition-contiguous scratch write for all of xw
            # (issued from the Activation queue so the SP queue keeps
            # streaming lsym)
            nc.scalar.dma_start(
                scratch[0:N, :].rearrange("(p a) f -> p a f", p=128),
                xw_all[:].rearrange("p (a f) -> p a f", a=KB),
            )

            # ---- phase A1 + GCN interleaved ----
            # gathers are issued gcn_delta kbs before their PE compute so the
            # in-order PE queue never head-of-line blocks on a gather DMA
            if gcn_sched is None:
                gcn_sched = [3 + 7 * i for i in range(MB)]
            assert len(gcn_sched) == MB and all(0 <= k < KB for k in gcn_sched)
            gcn_delta = 4
            gsched, csched = {}, {}
            for b, k in enumerate(gcn_sched):
                gsched.setdefault(k, []).append(b)
                csched.setdefault(min(k + gcn_delta, KB - 1), []).append(b)
            hhps = ps_big.tile([D, M], F32, tag="hh")
            ls_sb = None

            if NI < C * 128:
                # fallback path: tail slots of each group are never gathered;
                # pre-zero the gather destinations so stale NaNs can't leak
                # through zero seg weights
                for _ in range(2):
                    m0 = msg_pool.tile([128, QB * D], F32, tag="msg")
                    nc.vector.memset(m0[:], 0)

            pending = {}

            def emit_gcn_gather(b):
                seg_sb = seg_pool.tile([128, QB * G], DT, tag="seg")
                nc.sync.dma_start(seg_sb[:], segT[b * 128:(b + 1) * 128, :])
                msg_sb = msg_pool.tile([128, QB * D], F32, tag="msg")
                msgv3 = msg_sb[:].rearrange("p (c f) -> p c f", c=QB)
                icols = (NI + 15) // 16
                for g in range(GPB):
                    # one exact-count gather per group (NI <= 1024 fits the
                    # single_packet 64-descriptor/engine limit)
                    nc.gpsimd.dma_gather(
                        msgv3[:, g * C:(g + 1) * C, :],
                        scratch[:],
                        idx_sb[:, (b * GPB + g) * C * 8:
                               (b * GPB + g) * C * 8 + icols],
                        NI, NI, D,
                    )
                msgh_sb = msgh_pool.tile([128, QB * D], DT, tag="msgh")
                nc.vector.tensor_copy(msgh_sb[:], msg_sb[:])
                pending[b] = (seg_sb, msgh_sb)

            def emit_gcn_compute(b):
                seg_sb, msgh_sb = pending.pop(b)
                # self-loop + bias: hl_pre = xw_own * wself + bias
                hlp_sb = hlp_pool.tile([128, D], F32, tag="hlp")
                nc.vector.scalar_tensor_tensor(
                    hlp_sb[:], xw_all[:, b * D:(b + 1) * D],
                    wself_sb[:, b:b + 1], bias_sb[:],
                    ALU.mult, ALU.add)
                segv = seg_sb[:].rearrange("p (q t) -> p q t", t=G)
                msgv = msgh_sb[:].rearrange("p (c f) -> p c f", c=QB)
                for g in range(GPB):
                    hl = ps_small.tile([G, D], F32, tag="ps")
                    for c in range(C):
                        q = g * C + c
                        nc.tensor.matmul(
                            hl[:], lhsT=segv[:, q, :], rhs=msgv[:, q, :],
                            start=(c == 0), stop=(c == C - 1))
                    nc.vector.tensor_add(
                        hl_sb[G * g:G * (g + 1), b * D:(b + 1) * D],
                        hl[:], hlp_sb[G * g:G * (g + 1), :])

            for kb in range(KB):
                if kb % ls_pack == 0:
                    ls_sb = ls_pool.tile([128, ls_pack * M], lsdt, tag="ls")
                    r0 = kb * 128
                    nc.sync.dma_start(
                        ls_sb[:].rearrange("p (t m) -> p t m", t=ls_pack),
                        lsymT[r0:r0 + ls_pack * 128, :]
                        .rearrange("(t p) m -> p t m", p=128),
                    )
                lsv = ls_sb[:].rearrange("p (t m) -> p t m", t=ls_pack)
                for mc in range(M // mm_w):
                    nc.tensor.matmul(
                        hhps[:, mc * mm_w:(mc + 1) * mm_w],
                        lhsT=y_all[:, kb * D:(kb + 1) * D],
                        rhs=lsv[:, kb % ls_pack, mc * mm_w:(mc + 1) * mm_w],
                        start=(kb == 0), stop=(kb == KB - 1),
                    )
                if kb in gsched:
                    for b in gsched[kb]:
                        emit_gcn_gather(b)
                if kb in csched:
                    for b in csched[kb]:
                        emit_gcn_compute(b)

            # ---- final: per-block copy, transpose, combine; split store ----
            bpw = MB // out_split
            for b in range(MB):
                nc.vector.tensor_copy(hh_sb[:, b * 128:(b + 1) * 128],
                                      hhps[:, b * 128:(b + 1) * 128])
                ptile = ps_a0.tile([128, 8 * D], F32, tag="ps")
                pt = ptile[:, 0:D]
                nc.tensor.transpose(pt, hh_sb[:, b * 128:(b + 1) * 128],
                                    ident_sb[:])
                nc.vector.tensor_add(ob_sb[:, b * D:(b + 1) * D],
                                     hl_sb[:, b * D:(b + 1) * D], pt)
                if (b + 1) % bpw == 0:
                    b0 = b + 1 - bpw
                    nc.scalar.dma_start(
                        outp[b0 * 128:(b + 1) * 128, :]
                        .rearrange("(b p) f -> p b f", p=128),
                        ob_sb[:, b0 * D:(b + 1) * D]
                        .rearrange("p (b f) -> p b f", b=bpw),
                    )

    nc.compile()
    return nc


def _balance_targets(deg):
    """Assign each of N targets to (core, slot) so that every 32-target
    group has edge count as equal as possible. Returns (tgt_of_slot[NCORES,M],
    counts[NGRP]) where tgt_of_slot[j, b*128+q] = original node id."""
    ngrp = N // G
    per = E // ngrp  # 512
    order = np.argsort(-deg, kind="stable")
    # snake round-robin of degree-sorted targets over groups
    groups = np.empty((ngrp, G), np.int64)
    sums = np.zeros(ngrp, np.int64)
    for r in range(G):
        idx = order[r * ngrp:(r + 1) * ngrp]
        if r % 2 == 1:
            idx = idx[::-1]
        groups[:, r] = idx
        sums += deg[idx]
    # repair: swap targets between over/under groups to push sums to `per`
    for _ in range(50000):
        dev = int(np.abs(sums - per).sum())
        if dev == 0:
            break
        hi = int(np.argmax(sums))
        lo = int(np.argmin(sums))
        dh = deg[groups[hi]]
        dl = deg[groups[lo]]
        diff = dh[:, None] - dl[None, :]  # gain of swapping (a, b)
        # choose the swap minimizing the pair's total deviation afterward
        cost = (np.abs(sums[hi] - diff - per)
                + np.abs(sums[lo] + diff - per))
        pick = int(cost.argmin())
        a, b = pick // G, pick % G
        gain = int(diff[a, b])
        cur = abs(sums[hi] - per) + abs(sums[lo] - per)
        if int(cost[a, b]) >= cur or gain == 0:
            break  # no improving swap between the extreme pair
        groups[hi, a], groups[lo, b] = groups[lo, b], groups[hi, a]
        sums[hi] -= gain
        sums[lo] += gain
    tgt_of_slot = groups.reshape(NCORES, M)
    return tgt_of_slot, sums


def _prepare_host(x, edge_index, Lsym, W_high, W_conv, b_conv, aL, aH):
    """Shard + preprocess inputs. Returns (in_maps, C, NI, perm_back)."""
    x = np.asarray(x, np.float32)
    edge_index = np.asarray(edge_index)
    Lsym = np.asarray(Lsym, np.float32)
    W_high = np.asarray(W_high, np.float32)
    W_conv = np.asarray(W_conv, np.float32)
    b_conv = np.asarray(b_conv, np.float32)
    aL = float(np.asarray(aL))
    aH = float(np.asarray(aH))

    src_e = edge_index[0].astype(np.int64)
    tgt_e = edge_index[1].astype(np.int64)

    # degrees with self loops (matches PyG GCNConv gcn_norm)
    indeg = np.bincount(tgt_e, minlength=N).astype(np.int64)
    deg = indeg.astype(np.float64) + 1.0
    dinv = 1.0 / np.sqrt(deg)

    # ---- balanced target assignment ----
    tgt_of_slot, grp_sums = _balance_targets(indeg)  # [NCORES, M]
    NI = int(grp_sums.max())
    C = (NI + 127) // 128
    QB = GPB * C
    S = MB * QB * 128

    slot_of_tgt = np.empty(N, np.int64)       # global slot = core*M + m
    slot_of_tgt[tgt_of_slot.reshape(-1)] = np.arange(N)

    # per-core xT position: own targets (slot order) first, then the rest
    # pos_of_node[j, n] = position of node n in core j's xT
    pos_of_node = np.empty((NCORES, N), np.int64)
    all_nodes = np.arange(N, dtype=np.int64)
    for j in range(NCORES):
        own = tgt_of_slot[j]
        rest = np.setdiff1d(all_nodes, own, assume_unique=False)
        pos_of_node[j, own] = np.arange(M)
        pos_of_node[j, rest] = M + np.arange(N - M)

    # ---- per-edge placement ----
    w = (aL * dinv[src_e] * dinv[tgt_e]).astype(np.float32)
    gslot = slot_of_tgt[tgt_e]                 # global slot of the target
    core = gslot // M
    m = gslot % M                              # slot within core
    grp = gslot // G                           # global group id
    order = np.argsort(grp, kind="stable")
    srcs_o, w_o, core_o, m_o, grp_o = (src_e[order], w[order], core[order],
                                       m[order], grp[order])
    counts = np.bincount(grp, minlength=N // G)
    grp_start = np.zeros(N // G, np.int64)
    grp_start[1:] = np.cumsum(counts)[:-1]
    pos = np.arange(E) - grp_start[grp_o]      # position within group
    gic = grp_o % (M // G)                     # group index within core

    # gather index (scratch-row space per core), zero-row for padding
    pos_src = pos_of_node[core_o, srcs_o]
    scr_rows = ((pos_src % 128) * KB + pos_src // 128).astype(np.int16)
    slot = gic * (C * 128) + pos               # slot within the core
    gidx_all = np.full((NCORES, S), ZERO_ROW, np.int16)
    gidx_all[core_o, slot] = scr_rows

    # segment matrix, partition-contiguous layout:
    # row = block*128 + pos%128, col = (group-in-block*C + chunk)*G + q%G
    segT_all = np.zeros((NCORES, MB * 128, QB * G), NPDT)
    blk = gic // GPB
    qq = (gic % GPB) * C + pos // 128
    segT_all[core_o, blk * 128 + pos % 128, qq * G + m_o % G] = \
        w_o.astype(NPDT)

    wt2_scale = 1.0 / LS_SCALE if LS_MODE == "fp8" else 1.0
    wt2 = np.ascontiguousarray(np.concatenate(
        [W_high.T * wt2_scale, W_conv.T], axis=1)).astype(NPDT)
    bias128 = np.tile((aL * b_conv).astype(np.float32)[None, :], (128, 1))
    ident = np.eye(D, dtype=np.float32)

    wself_vals = (aL * dinv * dinv).astype(np.float32)  # per target node

    import ml_dtypes
    in_maps = []
    for j in range(NCORES):
        node_at = np.empty(N, np.int64)
        node_at[pos_of_node[j]] = all_nodes
        lsym_j = (aH * Lsym[tgt_of_slot[j]][:, node_at]).T
        if LS_MODE == "fp8":
            lsymT_j = np.ascontiguousarray(
                np.clip(lsym_j * LS_SCALE, -E3M4_MAX, E3M4_MAX)
                .astype(ml_dtypes.float8_e3m4))
        else:
            lsymT_j = np.ascontiguousarray(lsym_j.astype(NPDT))
        xT_j = np.ascontiguousarray(x.T[:, node_at]).astype(NPDT)
        g = gidx_all[j]
        gw = np.ascontiguousarray(g.reshape(S // 16, 16).T)  # [16, S/16]
        wself_j = np.ascontiguousarray(
            wself_vals[tgt_of_slot[j]].reshape(MB, 128).T)   # [128, MB]
        in_maps.append({
            "lsymT": lsymT_j,
            "xT": xT_j,
            "wt2": wt2,
            "segT": np.ascontiguousarray(segT_all[j]),
            "gidx": np.ascontiguousarray(np.tile(gw, (8, 1))),
            "bias128": bias128,
            "wself": wself_j,
            "ident": ident,
        })
    perm_back = slot_of_tgt  # out_full[t] = res[slot//M][slot%M]
    return in_maps, C, NI, perm_back


_CACHE = {}


def kernel(x, edge_index, Lsym, W_high, W_conv, b_conv, aL, aH):
    in_maps, C, NI, perm_back = _prepare_host(
        x, edge_index, Lsym, W_high, W_conv, b_conv, aL, aH)
    key = (C, NI, LS_MODE)
    nc = _CACHE.get(key)
    if nc is None:
        nc = _build_program(C, NI, LS_MODE)
        _CACHE[key] = nc
    res = run_bass_kernel_spmd(nc, in_maps, core_ids=list(range(NCORES)))
    out = np.concatenate([res.results[j]["out"] for j in range(NCORES)],
                         axis=0)
    return out[perm_back].astype(np.float32)


# revision 15
# speedup vs baseline: 1.0023x; 1.0023x over previous
"""FBGCN layer kernel for 8 Trainium2 NeuronCores.

out = aL * GCNConv(x, edge_index; W_conv, b_conv) + aH * (Lsym @ relu(x @ W_high.T))

Sharding: 1D row-partition of output nodes across 8 cores (1536 rows each),
with a host-chosen permutation of targets so that every 32-target group has
(nearly) the same number of incoming edges on every core. Host unpermutes
the concatenated result.

Each core:
  - computes Y = relu(x @ W_high.T)/s and xw = x @ W_conv.T for ALL nodes
    (x replicated, columns permuted per-core), writes xw to a DRAM scratch
    in a partition-contiguous layout,
  - streams its column slice of clip(s*aH*Lsym.T) (fp8 e3m4) through the PE
    with Y blocks stationary, accumulating HhT = (aH*Lsym_rows @ Y).T in
    PSUM (the 1/s is folded into W_high),
  - gathers per-edge source rows of xw from the scratch with dma_gather
    (one gather per 32-target group, exact counts - no padding), multiplies
    by a host-built sparse "segment matrix" (aL*norm weights folded in) on
    the PE to get the GCN edge aggregation per 128-target block (interleaved
    with the Lsym stream); self-loop terms are applied on-chip from xw
    directly (the per-core permutation puts own targets at scratch blocks
    0..MB-1),
  - transposes HhT blocks on the PE, adds everything and writes
    out[1536, 64] fp32.
No cross-core communication is needed.
"""

import numpy as np

import concourse.bacc as bacc
import concourse.mybir as mybir
import concourse.tile as tile
from concourse.bass_utils import run_bass_kernel_spmd

N, E, D = 12288, 196608, 64
NCORES = 8
M = N // NCORES          # 1536 output rows per core
MB = M // 128            # 12 target blocks per core
KB = N // 128            # 96 contraction blocks
ZERO_ROW = N             # scratch row of zeros (dummy gather target)
SCR_ROWS = N + 1
XT_CHUNK = 48            # kb blocks per xT DMA chunk (one load)
G = 32                   # targets per gather-group
GPB = 128 // G           # groups per 128-target block

DT = mybir.dt.float16
NPDT = np.float16
F32 = mybir.dt.float32
AFT = mybir.ActivationFunctionType
ALU = mybir.AluOpType

# high-pass (Lsym) stream dtype: "fp8" (e3m4, scaled) or "fp16"
LS_MODE = "fp8"
LS_SCALE = 512.0
E3M4_MAX = 15.5


def _ls_dt():
    return mybir.dt.float8e3 if LS_MODE == "fp8" else DT


def _scratch_row(pos):
    """xT position -> scratch row (partition-contiguous layout)."""
    return (pos % 128) * KB + pos // 128


def _build_program(C: int, NI: int, ls_mode: str, ls_pack=8, ls_bufs=3,
                   xt_chunk=XT_CHUNK, gcn_sched=None, mm_w=512,
                   out_split=2):
    """Build the SPMD Bass program.

    C = 128-slot chunks per G-target group; NI = gather index count per
    group (static, uniform across cores; NI <= C*128)."""
    QB = GPB * C            # chunks per 128-target block
    S = MB * QB * 128       # slot space per core (gidx layout)
    lsdt = mybir.dt.float8e3 if ls_mode == "fp8" else DT
    lsz = 1 if ls_mode == "fp8" else 2
    # keep SBUF in budget for denser graphs (fallback path)
    if C > 5:
        ls_bufs = 2
    if C > 8:
        ls_pack = 4
    nc = bacc.Bacc("TRN2", target_bir_lowering=False, debug=False,
                   num_devices=NCORES)

    lsymT = nc.dram_tensor("lsymT", [N, M], lsdt, kind="ExternalInput")
    xT = nc.dram_tensor("xT", [D, N], DT, kind="ExternalInput")
    wt2 = nc.dram_tensor("wt2", [D, 2 * D], DT, kind="ExternalInput")
    segT = nc.dram_tensor("segT", [MB * 128, QB * G], DT,
                          kind="ExternalInput")
    gidx = nc.dram_tensor("gidx", [128, S // 16], mybir.dt.int16,
                          kind="ExternalInput")
    bias128 = nc.dram_tensor("bias128", [128, D], F32, kind="ExternalInput")
    wself = nc.dram_tensor("wself", [128, MB], F32, kind="ExternalInput")
    ident = nc.dram_tensor("ident", [D, D], F32, kind="ExternalInput")
    outp = nc.dram_tensor("out", [M, D], F32, kind="ExternalOutput")

    with tile.TileContext(nc) as tc:
        with (
            tc.tile_pool(name="consts", bufs=1) as consts,
            tc.tile_pool(name="dram", bufs=1, space="DRAM") as dram,
            tc.tile_pool(name="xt", bufs=2) as xt_pool,
            tc.tile_pool(name="ls", bufs=ls_bufs) as ls_pool,
            tc.tile_pool(name="seg", bufs=3) as seg_pool,
            tc.tile_pool(name="msg", bufs=3) as msg_pool,
            tc.tile_pool(name="msgh", bufs=3) as msgh_pool,
            tc.tile_pool(name="hlp", bufs=2) as hlp_pool,
            tc.tile_pool(name="psb", bufs=1, space="PSUM") as ps_big,
            tc.tile_pool(name="psa", bufs=2, space="PSUM") as ps_a0,
            tc.tile_pool(name="pss", bufs=3, space="PSUM") as ps_small,
        ):
            # ---- constants / persistent tiles ----
            wt2_sb = consts.tile([D, 2 * D], DT, tag="wt2")
            nc.scalar.dma_start(wt2_sb[:], wt2[:])
            ident_sb = consts.tile([D, D], F32, tag="ident")
            nc.scalar.dma_start(ident_sb[:], ident[:])
            bias_sb = consts.tile([128, D], F32, tag="bias")
            nc.scalar.dma_start(bias_sb[:], bias128[:])
            wself_sb = consts.tile([128, MB], F32, tag="wself")
            nc.scalar.dma_start(wself_sb[:], wself[:])
            idx_sb = consts.tile([128, S // 16], mybir.dt.int16, tag="idx")
            nc.scalar.dma_start(idx_sb[:], gidx[:])
            zrow_sb = consts.tile([1, D], F32, tag="zrow")
            nc.vector.memset(zrow_sb[:], 0)
            y_all = consts.tile([128, KB * D], DT, tag="yall")
            xw_all = consts.tile([128, KB * D], F32, tag="xwall")
            hh_sb = consts.tile([D, M], F32, tag="hh")
            hl_sb = consts.tile([128, MB * D], F32, tag="hl")
            ob_sb = consts.tile([128, MB * D], F32, tag="ob")

            scratch = dram.tile([SCR_ROWS, D], F32, tag="scr")
            nc.scalar.dma_start(scratch[ZERO_ROW:ZERO_ROW + 1, :], zrow_sb[:])

            # ---- phase A0: Y = relu(x@Wh.T), xw = x@Wc.T for all nodes ----
            # four kb blocks per PSUM tile / activation / copy
            xt_sb = None
            for kb4 in range(KB // 4):
                kb = kb4 * 4
                if kb % xt_chunk == 0:
                    xt_sb = xt_pool.tile([D, xt_chunk * 128], DT, tag="xt")
                    c0 = kb * 128
                    nc.scalar.dma_start(xt_sb[:], xT[:, c0:c0 + xt_chunk * 128])
                ps = ps_a0.tile([128, 8 * D], F32, tag="ps")
                for h in range(4):
                    o = (kb + h) % xt_chunk
                    nc.tensor.matmul(
                        ps[:, h * 2 * D:(h + 1) * 2 * D],
                        lhsT=xt_sb[:, o * 128:(o + 1) * 128],
                        rhs=wt2_sb[:],
                        start=True, stop=True,
                    )
                psv = ps[:].rearrange("p (h c) -> p h c", h=4)
                nc.scalar.activation(
                    y_all[:, kb * D:(kb + 4) * D]
                    .rearrange("p (h c) -> p h c", h=4),
                    psv[:, :, 0:D], AFT.Relu)
                nc.vector.tensor_copy(
                    xw_all[:, kb * D:(kb + 4) * D]
                    .rearrange("p (h c) -> p h c", h=4),
                    psv[:, :, D:2 * D])
            # one partition-contiguous scratch write for all of xw
            # (issued from the Activation queue so the SP queue keeps
            # streaming lsym)
            nc.scalar.dma_start(
                scratch[0:N, :].rearrange("(p a) f -> p a f", p=128),
                xw_all[:].rearrange("p (a f) -> p a f", a=KB),
            )

            # ---- phase A1 + GCN interleaved ----
            # gathers are issued gcn_delta kbs before their PE compute so the
            # in-order PE queue never head-of-line blocks on a gather DMA
            if gcn_sched is None:
                gcn_sched = [3 + 7 * i for i in range(MB)]
            assert len(gcn_sched) == MB and all(0 <= k < KB for k in gcn_sched)
            gcn_delta = 4
            gsched, csched = {}, {}
            for b, k in enumerate(gcn_sched):
                gsched.setdefault(k, []).append(b)
                csched.setdefault(min(k + gcn_delta, KB - 1), []).append(b)
            hhps = ps_big.tile([D, M], F32, tag="hh")
            ls_sb = None

            if NI < C * 128:
                # fallback path: tail slots of each group are never gathered;
                # pre-zero the gather destinations so stale NaNs can't leak
                # through zero seg weights
                for _ in range(2):
                    m0 = msg_pool.tile([128, QB * D], F32, tag="msg")
                    nc.vector.memset(m0[:], 0)

            pending = {}

            def emit_gcn_gather(b):
                seg_sb = seg_pool.tile([128, QB * G], DT, tag="seg")
                nc.scalar.dma_start(seg_sb[:], segT[b * 128:(b + 1) * 128, :])
                msg_sb = msg_pool.tile([128, QB * D], F32, tag="msg")
                msgv3 = msg_sb[:].rearrange("p (c f) -> p c f", c=QB)
                icols = (NI + 15) // 16
                for g in range(GPB):
                    # one exact-count gather per group (NI <= 1024 fits the
                    # single_packet 64-descriptor/engine limit)
                    nc.gpsimd.dma_gather(
                        msgv3[:, g * C:(g + 1) * C, :],
                        scratch[:],
                        idx_sb[:, (b * GPB + g) * C * 8:
                               (b * GPB + g) * C * 8 + icols],
                        NI, NI, D,
                    )
                msgh_sb = msgh_pool.tile([128, QB * D], DT, tag="msgh")
                nc.vector.tensor_copy(msgh_sb[:], msg_sb[:])
                pending[b] = (seg_sb, msgh_sb)

            def emit_gcn_compute(b):
                seg_sb, msgh_sb = pending.pop(b)
                # self-loop + bias: hl_pre = xw_own * wself + bias
                hlp_sb = hlp_pool.tile([128, D], F32, tag="hlp")
                nc.vector.scalar_tensor_tensor(
                    hlp_sb[:], xw_all[:, b * D:(b + 1) * D],
                    wself_sb[:, b:b + 1], bias_sb[:],
                    ALU.mult, ALU.add)
                segv = seg_sb[:].rearrange("p (q t) -> p q t", t=G)
                msgv = msgh_sb[:].rearrange("p (c f) -> p c f", c=QB)
                for g in range(GPB):
                    hl = ps_small.tile([G, D], F32, tag="ps")
                    for c in range(C):
                        q = g * C + c
                        nc.tensor.matmul(
                            hl[:], lhsT=segv[:, q, :], rhs=msgv[:, q, :],
                            start=(c == 0), stop=(c == C - 1))
                    nc.vector.tensor_add(
                        hl_sb[G * g:G * (g + 1), b * D:(b + 1) * D],
                        hl[:], hlp_sb[G * g:G * (g + 1), :])

            for kb in range(KB):
                if kb % ls_pack == 0:
                    ls_sb = ls_pool.tile([128, ls_pack * M], lsdt, tag="ls")
                    r0 = kb * 128
                    nc.sync.dma_start(
                        ls_sb[:].rearrange("p (t m) -> p t m", t=ls_pack),
                        lsymT[r0:r0 + ls_pack * 128, :]
                        .rearrange("(t p) m -> p t m", p=128),
                    )
                lsv = ls_sb[:].rearrange("p (t m) -> p t m", t=ls_pack)
                for mc in range(M // mm_w):
                    nc.tensor.matmul(
                        hhps[:, mc * mm_w:(mc + 1) * mm_w],
                        lhsT=y_all[:, kb * D:(kb + 1) * D],
                        rhs=lsv[:, kb % ls_pack, mc * mm_w:(mc + 1) * mm_w],
                        start=(kb == 0), stop=(kb == KB - 1),
                    )
                if kb in gsched:
                    for b in gsched[kb]:
                        emit_gcn_gather(b)
                if kb in csched:
                    for b in csched[kb]:
                        emit_gcn_compute(b)

            # ---- final: per-block copy, transpose, combine; split store ----
            bpw = MB // out_split
            for b in range(MB):
                nc.vector.tensor_copy(hh_sb[:, b * 128:(b + 1) * 128],
                                      hhps[:, b * 128:(b + 1) * 128])
                ptile = ps_a0.tile([128, 8 * D], F32, tag="ps")
                pt = ptile[:, 0:D]
                nc.tensor.transpose(pt, hh_sb[:, b * 128:(b + 1) * 128],
                                    ident_sb[:])
                nc.vector.tensor_add(ob_sb[:, b * D:(b + 1) * D],
                                     hl_sb[:, b * D:(b + 1) * D], pt)
                if (b + 1) % bpw == 0:
                    b0 = b + 1 - bpw
                    nc.scalar.dma_start(
                        outp[b0 * 128:(b + 1) * 128, :]
                        .rearrange("(b p) f -> p b f", p=128),
                        ob_sb[:, b0 * D:(b + 1) * D]
                        .rearrange("p (b f) -> p b f", b=bpw),
                    )

    nc.compile()
    return nc


def _balance_targets(deg):
    """Assign each of N targets to (core, slot) so that every 32-target
    group has edge count as equal as possible. Returns (tgt_of_slot[NCORES,M],
    counts[NGRP]) where tgt_of_slot[j, b*128+q] = original node id."""
    ngrp = N // G
    per = E // ngrp  # 512
    order = np.argsort(-deg, kind="stable")
    # snake round-robin of degree-sorted targets over groups
    groups = np.empty((ngrp, G), np.int64)
    sums = np.zeros(ngrp, np.int64)
    for r in range(G):
        idx = order[r * ngrp:(r + 1) * ngrp]
        if r % 2 == 1:
            idx = idx[::-1]
        groups[:, r] = idx
        sums += deg[idx]
    # repair: swap targets between over/under groups to push sums to `per`
    for _ in range(50000):
        dev = int(np.abs(sums - per).sum())
        if dev == 0:
            break
        hi = int(np.argmax(sums))
        lo = int(np.argmin(sums))
        dh = deg[groups[hi]]
        dl = deg[groups[lo]]
        diff = dh[:, None] - dl[None, :]  # gain of swapping (a, b)
        # choose the swap minimizing the pair's total deviation afterward
        cost = (np.abs(sums[hi] - diff - per)
                + np.abs(sums[lo] + diff - per))
        pick = int(cost.argmin())
        a, b = pick // G, pick % G
        gain = int(diff[a, b])
        cur = abs(sums[hi] - per) + abs(sums[lo] - per)
        if int(cost[a, b]) >= cur or gain == 0:
            break  # no improving swap between the extreme pair
        groups[hi, a], groups[lo, b] = groups[lo, b], groups[hi, a]
        sums[hi] -= gain
        sums[lo] += gain
    tgt_of_slot = groups.reshape(NCORES, M)
    return tgt_of_slot, sums


def _prepare_host(x, edge_index, Lsym, W_high, W_conv, b_conv, aL, aH):
    """Shard + preprocess inputs. Returns (in_maps, C, NI, perm_back)."""
    x = np.asarray(x, np.float32)
    edge_index = np.asarray(edge_index)
    Lsym = np.asarray(Lsym, np.float32)
    W_high = np.asarray(W_high, np.float32)
    W_conv = np.asarray(W_conv, np.float32)
    b_conv = np.asarray(b_conv, np.float32)
    aL = float(np.asarray(aL))
    aH = float(np.asarray(aH))

    src_e = edge_index[0].astype(np.int64)
    tgt_e = edge_index[1].astype(np.int64)

    # degrees with self loops (matches PyG GCNConv gcn_norm)
    indeg = np.bincount(tgt_e, minlength=N).astype(np.int64)
    deg = indeg.astype(np.float64) + 1.0
    dinv = 1.0 / np.sqrt(deg)

    # ---- balanced target assignment ----
    tgt_of_slot, grp_sums = _balance_targets(indeg)  # [NCORES, M]
    NI = int(grp_sums.max())
    C = (NI + 127) // 128
    QB = GPB * C
    S = MB * QB * 128

    slot_of_tgt = np.empty(N, np.int64)       # global slot = core*M + m
    slot_of_tgt[tgt_of_slot.reshape(-1)] = np.arange(N)

    # per-core xT position: own targets (slot order) first, then the rest
    # pos_of_node[j, n] = position of node n in core j's xT
    pos_of_node = np.empty((NCORES, N), np.int64)
    all_nodes = np.arange(N, dtype=np.int64)
    for j in range(NCORES):
        own = tgt_of_slot[j]
        rest = np.setdiff1d(all_nodes, own, assume_unique=False)
        pos_of_node[j, own] = np.arange(M)
        pos_of_node[j, rest] = M + np.arange(N - M)

    # ---- per-edge placement ----
    w = (aL * dinv[src_e] * dinv[tgt_e]).astype(np.float32)
    gslot = slot_of_tgt[tgt_e]                 # global slot of the target
    core = gslot // M
    m = gslot % M                              # slot within core
    grp = gslot // G                           # global group id
    order = np.argsort(grp, kind="stable")
    srcs_o, w_o, core_o, m_o, grp_o = (src_e[order], w[order], core[order],
                                       m[order], grp[order])
    counts = np.bincount(grp, minlength=N // G)
    grp_start = np.zeros(N // G, np.int64)
    grp_start[1:] = np.cumsum(counts)[:-1]
    pos = np.arange(E) - grp_start[grp_o]      # position within group
    gic = grp_o % (M // G)                     # group index within core

    # gather index (scratch-row space per core), zero-row for padding
    pos_src = pos_of_node[core_o, srcs_o]
    scr_rows = ((pos_src % 128) * KB + pos_src // 128).astype(np.int16)
    slot = gic * (C * 128) + pos               # slot within the core
    gidx_all = np.full((NCORES, S), ZERO_ROW, np.int16)
    gidx_all[core_o, slot] = scr_rows

    # segment matrix, partition-contiguous layout:
    # row = block*128 + pos%128, col = (group-in-block*C + chunk)*G + q%G
    segT_all = np.zeros((NCORES, MB * 128, QB * G), NPDT)
    blk = gic // GPB
    qq = (gic % GPB) * C + pos // 128
    segT_all[core_o, blk * 128 + pos % 128, qq * G + m_o % G] = \
        w_o.astype(NPDT)

    wt2_scale = 1.0 / LS_SCALE if LS_MODE == "fp8" else 1.0
    wt2 = np.ascontiguousarray(np.concatenate(
        [W_high.T * wt2_scale, W_conv.T], axis=1)).astype(NPDT)
    bias128 = np.tile((aL * b_conv).astype(np.float32)[None, :], (128, 1))
    ident = np.eye(D, dtype=np.float32)

    wself_vals = (aL * dinv * dinv).astype(np.float32)  # per target node

    import ml_dtypes
    in_maps = []
    for j in range(NCORES):
        node_at = np.empty(N, np.int64)
        node_at[pos_of_node[j]] = all_nodes
        lsym_j = (aH * Lsym[tgt_of_slot[j]][:, node_at]).T
        if LS_MODE == "fp8":
            lsymT_j = np.ascontiguousarray(
                np.clip(lsym_j * LS_SCALE, -E3M4_MAX, E3M4_MAX)
                .astype(ml_dtypes.float8_e3m4))
        else:
            lsymT_j = np.ascontiguousarray(lsym_j.astype(NPDT))
        xT_j = np.ascontiguousarray(x.T[:, node_at]).astype(NPDT)
        g = gidx_all[j]
        gw = np.ascontiguousarray(g.reshape(S // 16, 16).T)  # [16, S/16]
        wself_j = np.ascontiguousarray(
            wself_vals[tgt_of_slot[j]].reshape(MB, 128).T)   # [128, MB]
        in_maps.append({
            "lsymT": lsymT_j,
            "xT": xT_j,
            "wt2": wt2,
            "segT": np.ascontiguousarray(segT_all[j]),
            "gidx": np.ascontiguousarray(np.tile(gw, (8, 1))),
            "bias128": bias128,
            "wself": wself_j,
            "ident": ident,
        })
    perm_back = slot_of_tgt  # out_full[t] = res[slot//M][slot%M]
    return in_maps, C, NI, perm_back


_CACHE = {}


def kernel(x, edge_index, Lsym, W_high, W_conv, b_conv, aL, aH):
    in_maps, C, NI, perm_back = _prepare_host(
        x, edge_index, Lsym, W_high, W_conv, b_conv, aL, aH)
    key = (C, NI, LS_MODE)
    nc = _CACHE.get(key)
    if nc is None:
        nc = _build_program(C, NI, LS_MODE)
        _CACHE[key] = nc
    res = run_bass_kernel_spmd(nc, in_maps, core_ids=list(range(NCORES)))
    out = np.concatenate([res.results[j]["out"] for j in range(NCORES)],
                         axis=0)
    return out[perm_back].astype(np.float32)


# revision 17
# speedup vs baseline: 1.0391x; 1.0368x over previous
"""FBGCN layer kernel for 8 Trainium2 NeuronCores.

out = aL * GCNConv(x, edge_index; W_conv, b_conv) + aH * (Lsym @ relu(x @ W_high.T))

Sharding: 1D row-partition of output nodes across 8 cores (1536 rows each),
with a host-chosen permutation of targets so that every 32-target group has
(nearly) the same number of incoming edges on every core. Host unpermutes
the concatenated result.

Each core:
  - computes Y = relu(x @ W_high.T)/s and xw = x @ W_conv.T for ALL nodes
    (x replicated, columns permuted per-core), writes xw to a DRAM scratch
    in a partition-contiguous layout,
  - streams its column slice of clip(s*aH*Lsym.T) (fp8 e3m4) through the PE
    with Y blocks stationary, accumulating HhT = (aH*Lsym_rows @ Y).T in
    PSUM (the 1/s is folded into W_high),
  - gathers per-edge source rows of xw from the scratch with dma_gather
    (one gather per 32-target group, exact counts - no padding), multiplies
    by a host-built sparse "segment matrix" (aL*norm weights folded in) on
    the PE to get the GCN edge aggregation per 128-target block (interleaved
    with the Lsym stream); self-loop terms are applied on-chip from xw
    directly (the per-core permutation puts own targets at scratch blocks
    0..MB-1),
  - transposes HhT blocks on the PE, adds everything and writes
    out[1536, 64] fp32.
No cross-core communication is needed.
"""

import numpy as np

import concourse.bacc as bacc
import concourse.mybir as mybir
import concourse.tile as tile
from concourse.bass_utils import run_bass_kernel_spmd

N, E, D = 12288, 196608, 64
NCORES = 8
M = N // NCORES          # 1536 output rows per core
MB = M // 128            # 12 target blocks per core
KB = N // 128            # 96 contraction blocks
ZERO_ROW = N             # scratch row of zeros (dummy gather target)
SCR_ROWS = N + 1
XT_CHUNK = 48            # kb blocks per xT DMA chunk (one load)
G = 32                   # targets per gather-group
GPB = 128 // G           # groups per 128-target block

DT = mybir.dt.float16
NPDT = np.float16
F32 = mybir.dt.float32
AFT = mybir.ActivationFunctionType
ALU = mybir.AluOpType

# high-pass (Lsym) stream dtype: "fp8" (e3m4, scaled) or "fp16"
LS_MODE = "fp8"
LS_SCALE = 512.0
E3M4_MAX = 15.5


def _ls_dt():
    return mybir.dt.float8e3 if LS_MODE == "fp8" else DT


def _scratch_row(pos):
    """xT position -> scratch row (partition-contiguous layout)."""
    return (pos % 128) * KB + pos // 128


def _build_program(C: int, NI: int, ls_mode: str, ls_pack=8, ls_bufs=3,
                   xt_chunk=XT_CHUNK, gcn_sched=None, mm_w=512,
                   out_split=2):
    """Build the SPMD Bass program.

    C = 128-slot chunks per G-target group; NI = gather index count per
    group (static, uniform across cores; NI <= C*128)."""
    QB = GPB * C            # chunks per 128-target block
    S = MB * QB * 128       # slot space per core (gidx layout)
    lsdt = mybir.dt.float8e3 if ls_mode == "fp8" else DT
    lsz = 1 if ls_mode == "fp8" else 2
    # keep SBUF in budget for denser graphs (fallback path)
    if C > 5:
        ls_bufs = 2
    if C > 8:
        ls_pack = 4
    nc = bacc.Bacc("TRN2", target_bir_lowering=False, debug=False,
                   num_devices=NCORES)

    lsymT = nc.dram_tensor("lsymT", [N, M], lsdt, kind="ExternalInput")
    xT = nc.dram_tensor("xT", [D, N], DT, kind="ExternalInput")
    wt2 = nc.dram_tensor("wt2", [D, 2 * D], DT, kind="ExternalInput")
    segT = nc.dram_tensor("segT", [MB * 128, QB * G], DT,
                          kind="ExternalInput")
    gidx = nc.dram_tensor("gidx", [128, S // 16], mybir.dt.int16,
                          kind="ExternalInput")
    bias128 = nc.dram_tensor("bias128", [128, D], F32, kind="ExternalInput")
    wself = nc.dram_tensor("wself", [128, MB], F32, kind="ExternalInput")
    ident = nc.dram_tensor("ident", [D, D], F32, kind="ExternalInput")
    outp = nc.dram_tensor("out", [M, D], F32, kind="ExternalOutput")

    with tile.TileContext(nc) as tc:
        with (
            tc.tile_pool(name="consts", bufs=1) as consts,
            tc.tile_pool(name="dram", bufs=1, space="DRAM") as dram,
            tc.tile_pool(name="xt", bufs=2) as xt_pool,
            tc.tile_pool(name="ls", bufs=ls_bufs) as ls_pool,
            tc.tile_pool(name="seg", bufs=3) as seg_pool,
            tc.tile_pool(name="msg", bufs=3) as msg_pool,
            tc.tile_pool(name="msgh", bufs=3) as msgh_pool,
            tc.tile_pool(name="hlp", bufs=2) as hlp_pool,
            tc.tile_pool(name="psb", bufs=1, space="PSUM") as ps_big,
            tc.tile_pool(name="psa", bufs=2, space="PSUM") as ps_a0,
            tc.tile_pool(name="pss", bufs=3, space="PSUM") as ps_small,
        ):
            # ---- constants / persistent tiles ----
            wt2_sb = consts.tile([D, 2 * D], DT, tag="wt2")
            nc.scalar.dma_start(wt2_sb[:], wt2[:])
            ident_sb = consts.tile([D, D], F32, tag="ident")
            nc.scalar.dma_start(ident_sb[:], ident[:])
            bias_sb = consts.tile([128, D], F32, tag="bias")
            nc.scalar.dma_start(bias_sb[:], bias128[:])
            wself_sb = consts.tile([128, MB], F32, tag="wself")
            nc.scalar.dma_start(wself_sb[:], wself[:])
            idx_sb = consts.tile([128, S // 16], mybir.dt.int16, tag="idx")
            nc.scalar.dma_start(idx_sb[:], gidx[:])
            zrow_sb = consts.tile([1, D], F32, tag="zrow")
            nc.vector.memset(zrow_sb[:], 0)
            y_all = consts.tile([128, KB * D], DT, tag="yall")
            xw_all = consts.tile([128, KB * D], F32, tag="xwall")
            hh_sb = consts.tile([D, M], F32, tag="hh")
            hl_sb = consts.tile([128, MB * D], F32, tag="hl")
            ob_sb = consts.tile([128, MB * D], F32, tag="ob")

            scratch = dram.tile([SCR_ROWS, D], F32, tag="scr")
            nc.scalar.dma_start(scratch[ZERO_ROW:ZERO_ROW + 1, :], zrow_sb[:])

            # ---- phase A0: Y = relu(x@Wh.T), xw = x@Wc.T for all nodes ----
            # four kb blocks per PSUM tile / activation / copy
            xt_sb = None
            for kb4 in range(KB // 4):
                kb = kb4 * 4
                if kb % xt_chunk == 0:
                    xt_sb = xt_pool.tile([D, xt_chunk * 128], DT, tag="xt")
                    c0 = kb * 128
                    nc.scalar.dma_start(xt_sb[:], xT[:, c0:c0 + xt_chunk * 128])
                ps = ps_a0.tile([128, 8 * D], F32, tag="ps")
                for h in range(4):
                    o = (kb + h) % xt_chunk
                    nc.tensor.matmul(
                        ps[:, h * 2 * D:(h + 1) * 2 * D],
                        lhsT=xt_sb[:, o * 128:(o + 1) * 128],
                        rhs=wt2_sb[:],
                        start=True, stop=True,
                    )
                psv = ps[:].rearrange("p (h c) -> p h c", h=4)
                yv = y_all[:, kb * D:(kb + 4) * D] \
                    .rearrange("p (h c) -> p h c", h=4)
                if kb4 % 2 == 0:
                    nc.scalar.activation(yv, psv[:, :, 0:D], AFT.Relu)
                else:
                    # alternate relu onto DVE so A0 isn't Act-bound
                    nc.vector.tensor_scalar_max(yv, psv[:, :, 0:D], 0.0)
                nc.vector.tensor_copy(
                    xw_all[:, kb * D:(kb + 4) * D]
                    .rearrange("p (h c) -> p h c", h=4),
                    psv[:, :, D:2 * D])
            # one partition-contiguous scratch write for all of xw
            # (issued from the Activation queue so the SP queue keeps
            # streaming lsym)
            nc.scalar.dma_start(
                scratch[0:N, :].rearrange("(p a) f -> p a f", p=128),
                xw_all[:].rearrange("p (a f) -> p a f", a=KB),
            )

            # ---- phase A1 + GCN interleaved ----
            # gathers are issued gcn_delta kbs before their PE compute so the
            # in-order PE queue never head-of-line blocks on a gather DMA
            if gcn_sched is None:
                gcn_sched = [3 + 7 * i for i in range(MB)]
            assert len(gcn_sched) == MB and all(0 <= k < KB for k in gcn_sched)
            gcn_delta = 4
            gsched, csched = {}, {}
            for b, k in enumerate(gcn_sched):
                gsched.setdefault(k, []).append(b)
                csched.setdefault(min(k + gcn_delta, KB - 1), []).append(b)
            hhps = ps_big.tile([D, M], F32, tag="hh")
            ls_sb = None

            if NI < C * 128:
                # fallback path: tail slots of each group are never gathered;
                # pre-zero the gather destinations so stale NaNs can't leak
                # through zero seg weights
                for _ in range(2):
                    m0 = msg_pool.tile([128, QB * D], F32, tag="msg")
                    nc.vector.memset(m0[:], 0)

            pending = {}

            def emit_gcn_gather(b):
                seg_sb = seg_pool.tile([128, QB * G], DT, tag="seg")
                nc.scalar.dma_start(seg_sb[:], segT[b * 128:(b + 1) * 128, :])
                msg_sb = msg_pool.tile([128, QB * D], F32, tag="msg")
                msgv3 = msg_sb[:].rearrange("p (c f) -> p c f", c=QB)
                if NI == C * 128 and GPB % 2 == 0:
                    # groups are exactly full: gather 2 adjacent groups per
                    # instruction (1024 idx = the single_packet limit)
                    for g2 in range(GPB // 2):
                        nidx = 2 * NI
                        nc.gpsimd.dma_gather(
                            msgv3[:, g2 * 2 * C:(g2 + 1) * 2 * C, :],
                            scratch[:],
                            idx_sb[:, (b * GPB + g2 * 2) * C * 8:
                                   (b * GPB + g2 * 2 + 2) * C * 8],
                            nidx, nidx, D,
                        )
                else:
                    icols = (NI + 15) // 16
                    for g in range(GPB):
                        # one exact-count gather per group (NI <= 1024 fits
                        # the single_packet 64-descriptor/engine limit)
                        nc.gpsimd.dma_gather(
                            msgv3[:, g * C:(g + 1) * C, :],
                            scratch[:],
                            idx_sb[:, (b * GPB + g) * C * 8:
                                   (b * GPB + g) * C * 8 + icols],
                            NI, NI, D,
                        )
                msgh_sb = msgh_pool.tile([128, QB * D], DT, tag="msgh")
                nc.vector.tensor_copy(msgh_sb[:], msg_sb[:])
                pending[b] = (seg_sb, msgh_sb)

            def emit_gcn_compute(b):
                seg_sb, msgh_sb = pending.pop(b)
                # self-loop + bias: hl_pre = xw_own * wself + bias
                hlp_sb = hlp_pool.tile([128, D], F32, tag="hlp")
                nc.vector.scalar_tensor_tensor(
                    hlp_sb[:], xw_all[:, b * D:(b + 1) * D],
                    wself_sb[:, b:b + 1], bias_sb[:],
                    ALU.mult, ALU.add)
                segv = seg_sb[:].rearrange("p (q t) -> p q t", t=G)
                msgv = msgh_sb[:].rearrange("p (c f) -> p c f", c=QB)
                for g in range(GPB):
                    hl = ps_small.tile([G, D], F32, tag="ps")
                    for c in range(C):
                        q = g * C + c
                        nc.tensor.matmul(
                            hl[:], lhsT=segv[:, q, :], rhs=msgv[:, q, :],
                            start=(c == 0), stop=(c == C - 1))
                    nc.vector.tensor_add(
                        hl_sb[G * g:G * (g + 1), b * D:(b + 1) * D],
                        hl[:], hlp_sb[G * g:G * (g + 1), :])

            for kb in range(KB):
                if kb % ls_pack == 0:
                    ls_sb = ls_pool.tile([128, ls_pack * M], lsdt, tag="ls")
                    r0 = kb * 128
                    nc.sync.dma_start(
                        ls_sb[:].rearrange("p (t m) -> p t m", t=ls_pack),
                        lsymT[r0:r0 + ls_pack * 128, :]
                        .rearrange("(t p) m -> p t m", p=128),
                    )
                lsv = ls_sb[:].rearrange("p (t m) -> p t m", t=ls_pack)
                for mc in range(M // mm_w):
                    nc.tensor.matmul(
                        hhps[:, mc * mm_w:(mc + 1) * mm_w],
                        lhsT=y_all[:, kb * D:(kb + 1) * D],
                        rhs=lsv[:, kb % ls_pack, mc * mm_w:(mc + 1) * mm_w],
                        start=(kb == 0), stop=(kb == KB - 1),
                    )
                if kb in gsched:
                    for b in gsched[kb]:
                        emit_gcn_gather(b)
                if kb in csched:
                    for b in csched[kb]:
                        emit_gcn_compute(b)

            # ---- final: per-block copy, transpose, combine; split store ----
            bpw = MB // out_split
            for b in range(MB):
                nc.vector.tensor_copy(hh_sb[:, b * 128:(b + 1) * 128],
                                      hhps[:, b * 128:(b + 1) * 128])
                ptile = ps_a0.tile([128, 8 * D], F32, tag="ps")
                pt = ptile[:, 0:D]
                nc.tensor.transpose(pt, hh_sb[:, b * 128:(b + 1) * 128],
                                    ident_sb[:])
                nc.vector.tensor_add(ob_sb[:, b * D:(b + 1) * D],
                                     hl_sb[:, b * D:(b + 1) * D], pt)
                if (b + 1) % bpw == 0:
                    b0 = b + 1 - bpw
                    nc.scalar.dma_start(
                        outp[b0 * 128:(b + 1) * 128, :]
                        .rearrange("(b p) f -> p b f", p=128),
                        ob_sb[:, b0 * D:(b + 1) * D]
                        .rearrange("p (b f) -> p b f", b=bpw),
                    )

    nc.compile()
    return nc


def _balance_targets(deg):
    """Assign each of N targets to (core, slot) so that every 32-target
    group has edge count as equal as possible. Returns (tgt_of_slot[NCORES,M],
    counts[NGRP]) where tgt_of_slot[j, b*128+q] = original node id."""
    ngrp = N // G
    per = E // ngrp  # 512
    order = np.argsort(-deg, kind="stable")
    # snake round-robin of degree-sorted targets over groups
    groups = np.empty((ngrp, G), np.int64)
    sums = np.zeros(ngrp, np.int64)
    for r in range(G):
        idx = order[r * ngrp:(r + 1) * ngrp]
        if r % 2 == 1:
            idx = idx[::-1]
        groups[:, r] = idx
        sums += deg[idx]
    # repair: swap targets between over/under groups to push sums to `per`
    for _ in range(50000):
        dev = int(np.abs(sums - per).sum())
        if dev == 0:
            break
        hi = int(np.argmax(sums))
        lo = int(np.argmin(sums))
        dh = deg[groups[hi]]
        dl = deg[groups[lo]]
        diff = dh[:, None] - dl[None, :]  # gain of swapping (a, b)
        # choose the swap minimizing the pair's total deviation afterward
        cost = (np.abs(sums[hi] - diff - per)
                + np.abs(sums[lo] + diff - per))
        pick = int(cost.argmin())
        a, b = pick // G, pick % G
        gain = int(diff[a, b])
        cur = abs(sums[hi] - per) + abs(sums[lo] - per)
        if int(cost[a, b]) >= cur or gain == 0:
            break  # no improving swap between the extreme pair
        groups[hi, a], groups[lo, b] = groups[lo, b], groups[hi, a]
        sums[hi] -= gain
        sums[lo] += gain
    tgt_of_slot = groups.reshape(NCORES, M)
    return tgt_of_slot, sums


def _prepare_host(x, edge_index, Lsym, W_high, W_conv, b_conv, aL, aH):
    """Shard + preprocess inputs. Returns (in_maps, C, NI, perm_back)."""
    x = np.asarray(x, np.float32)
    edge_index = np.asarray(edge_index)
    Lsym = np.asarray(Lsym, np.float32)
    W_high = np.asarray(W_high, np.float32)
    W_conv = np.asarray(W_conv, np.float32)
    b_conv = np.asarray(b_conv, np.float32)
    aL = float(np.asarray(aL))
    aH = float(np.asarray(aH))

    src_e = edge_index[0].astype(np.int64)
    tgt_e = edge_index[1].astype(np.int64)

    # degrees with self loops (matches PyG GCNConv gcn_norm)
    indeg = np.bincount(tgt_e, minlength=N).astype(np.int64)
    deg = indeg.astype(np.float64) + 1.0
    dinv = 1.0 / np.sqrt(deg)

    # ---- balanced target assignment ----
    tgt_of_slot, grp_sums = _balance_targets(indeg)  # [NCORES, M]
    NI = int(grp_sums.max())
    C = (NI + 127) // 128
    QB = GPB * C
    S = MB * QB * 128

    slot_of_tgt = np.empty(N, np.int64)       # global slot = core*M + m
    slot_of_tgt[tgt_of_slot.reshape(-1)] = np.arange(N)

    # per-core xT position: own targets (slot order) first, then the rest
    # pos_of_node[j, n] = position of node n in core j's xT
    pos_of_node = np.empty((NCORES, N), np.int64)
    all_nodes = np.arange(N, dtype=np.int64)
    for j in range(NCORES):
        own = tgt_of_slot[j]
        rest = np.setdiff1d(all_nodes, own, assume_unique=False)
        pos_of_node[j, own] = np.arange(M)
        pos_of_node[j, rest] = M + np.arange(N - M)

    # ---- per-edge placement ----
    w = (aL * dinv[src_e] * dinv[tgt_e]).astype(np.float32)
    gslot = slot_of_tgt[tgt_e]                 # global slot of the target
    core = gslot // M
    m = gslot % M                              # slot within core
    grp = gslot // G                           # global group id
    order = np.argsort(grp, kind="stable")
    srcs_o, w_o, core_o, m_o, grp_o = (src_e[order], w[order], core[order],
                                       m[order], grp[order])
    counts = np.bincount(grp, minlength=N // G)
    grp_start = np.zeros(N // G, np.int64)
    grp_start[1:] = np.cumsum(counts)[:-1]
    pos = np.arange(E) - grp_start[grp_o]      # position within group
    gic = grp_o % (M // G)                     # group index within core

    # gather index (scratch-row space per core), zero-row for padding
    pos_src = pos_of_node[core_o, srcs_o]
    scr_rows = ((pos_src % 128) * KB + pos_src // 128).astype(np.int16)
    slot = gic * (C * 128) + pos               # slot within the core
    gidx_all = np.full((NCORES, S), ZERO_ROW, np.int16)
    gidx_all[core_o, slot] = scr_rows

    # segment matrix, partition-contiguous layout:
    # row = block*128 + pos%128, col = (group-in-block*C + chunk)*G + q%G
    segT_all = np.zeros((NCORES, MB * 128, QB * G), NPDT)
    blk = gic // GPB
    qq = (gic % GPB) * C + pos // 128
    segT_all[core_o, blk * 128 + pos % 128, qq * G + m_o % G] = \
        w_o.astype(NPDT)

    wt2_scale = 1.0 / LS_SCALE if LS_MODE == "fp8" else 1.0
    wt2 = np.ascontiguousarray(np.concatenate(
        [W_high.T * wt2_scale, W_conv.T], axis=1)).astype(NPDT)
    bias128 = np.tile((aL * b_conv).astype(np.float32)[None, :], (128, 1))
    ident = np.eye(D, dtype=np.float32)

    wself_vals = (aL * dinv * dinv).astype(np.float32)  # per target node

    import ml_dtypes
    in_maps = []
    for j in range(NCORES):
        node_at = np.empty(N, np.int64)
        node_at[pos_of_node[j]] = all_nodes
        lsym_j = (aH * Lsym[tgt_of_slot[j]][:, node_at]).T
        if LS_MODE == "fp8":
            lsymT_j = np.ascontiguousarray(
                np.clip(lsym_j * LS_SCALE, -E3M4_MAX, E3M4_MAX)
                .astype(ml_dtypes.float8_e3m4))
        else:
            lsymT_j = np.ascontiguousarray(lsym_j.astype(NPDT))
        xT_j = np.ascontiguousarray(x.T[:, node_at]).astype(NPDT)
        g = gidx_all[j]
        gw = np.ascontiguousarray(g.reshape(S // 16, 16).T)  # [16, S/16]
        wself_j = np.ascontiguousarray(
            wself_vals[tgt_of_slot[j]].reshape(MB, 128).T)   # [128, MB]
        in_maps.append({
            "lsymT": lsymT_j,
            "xT": xT_j,
            "wt2": wt2,
            "segT": np.ascontiguousarray(segT_all[j]),
            "gidx": np.ascontiguousarray(np.tile(gw, (8, 1))),
            "bias128": bias128,
            "wself": wself_j,
            "ident": ident,
        })
    perm_back = slot_of_tgt  # out_full[t] = res[slot//M][slot%M]
    return in_maps, C, NI, perm_back


_CACHE = {}


def kernel(x, edge_index, Lsym, W_high, W_conv, b_conv, aL, aH):
    in_maps, C, NI, perm_back = _prepare_host(
        x, edge_index, Lsym, W_high, W_conv, b_conv, aL, aH)
    key = (C, NI, LS_MODE)
    nc = _CACHE.get(key)
    if nc is None:
        nc = _build_program(C, NI, LS_MODE)
        _CACHE[key] = nc
    res = run_bass_kernel_spmd(nc, in_maps, core_ids=list(range(NCORES)))
    out = np.concatenate([res.results[j]["out"] for j in range(NCORES)],
                         axis=0)
    return out[perm_back].astype(np.float32)
